# revision 54
# baseline (speedup 1.0000x reference)
"""Trainium2 Bass kernel v5 for nn_Encoder (B=1M, data-parallel on 8 cores).

Input stream packed 3-blocks-per-2-tiles (block 3k+0: tile 2k rows
0-84; 3k+1: split across tiles 2k/2k+1 at f=43; 3k+2: tile 2k+1 rows
42-126): same 11 tiles per 16-block group as an 88-row-slot layout but
only 5 tile-crossing blocks instead of 10, so L1 is 21 instead of 26
M=64 pair-streams per group.  248 blocks of 512 samples per core (1.6%
batch pad): 15 full groups + one 8-block tail group (6 stream tiles).

All matmuls are merged M=64 single instructions (L1 per stream-part,
L2 per pair, L3 per quad, heads per oct) with zero-padded lhsT columns
so every psum row is PE-written (no psum memsets).  relu1 operates on
wide [128,1024] psum tiles (2 pairs), halving its instruction count.
psum->sbuf passes are cost-weighted between DVE and ACT.  PE is the
bottleneck: ~(21+14)/2 pair-streams x ~255ns x 15.5 groups.

PSUM budget: ps1 2x[128,1024] (4 banks) + ps2 2x[128,512] (2) +
ps3 1x[128,512] + ps4 1x[128,512] = 8 banks.

Measured dead ends: 42-strip 4-way M=32 chunked L1 (slower than the
2-wide M=64 form), mixed M=64/M=32 accumulation groups (1.5x slower),
bf16 PSUM (TRN3-only), fp8 inputs (fails the 2e-2 gate at 0.028),
out-DMAs on the ACT HWDGE ring (stalls ACT's in-order sequencer),
deepening the relu2->L3 lag (-9.5us/pass) or relu3->heads lag
(-18.5us/pass) - only the relu1->L2 hop (double-buffered wide ps1,
6-deep h1 ring) rewards lag depth; downstream hops with single-
buffered ps3/ps4 rings punish it.  3-deep input prefetch (x pool
bufs=3, fetch two groups ahead) is far worse than 2-deep.  Pinning
Identity (out) passes to DVE is neutral (-1.6us, noise): no ACT
activation-table switch cost between Relu and Identity.  Extra SBUF
pool slack (h2/h3/osb +1 buf each) is -3.1us: beyond-necessary
buffering consistently hurts slightly (also seen with h1 8-buf and
3-deep x prefetch).
"""
import numpy as np
import ml_dtypes

import concourse.bass as bass
import concourse.mybir as mybir
import concourse.tile as tile
from concourse.bass_utils import run_bass_kernel_spmd

AF = mybir.ActivationFunctionType
ALU = mybir.AluOpType
F32 = mybir.dt.float32
BF16 = mybir.dt.bfloat16
BF16_NP = ml_dtypes.bfloat16

N_CORES = 8
B_FULL = 1_000_000
PER_CORE = B_FULL // N_CORES          # 125000
NGRP = 16                             # groups (15 full + 1 tail)
BLKS_PER_GRP = 16
TAIL_BLKS = 8
NBLK = 15 * BLKS_PER_GRP + TAIL_BLKS  # 248 blocks of 512
NTOT = NBLK * 512                     # 126976 padded samples/core
NOUT = 16                             # od tiles (tile 15 half garbage)
SLOT = 88                             # stream rows per block slot
GCOLS = 11 * 512                      # stream cols per full group
TCOLS = 6 * 512                       # stream cols of tail group
XCOLS = 15 * GCOLS + TCOLS            # 87552
NF = 85

# ---- L1 static plan ------------------------------------------------------
# feature f -> (j, k): j = f//5, k = f%5 (k<3: x, else c)
# out m -> (j2, o): j2 = m//3, o = m%3;  W1[o, k] if j == j2
# One merged M=64 instruction per stream-part; part 1 exists iff the
# 85-row sample window crosses the 128-row stream tile (p > 43).


def _l1_plan():
    """Per phase: ordered list of (tile_idx, shift, start, stop).

    3-blocks-per-2-tiles stream packing: block 3k+0 at tile 2k rows
    0-84, block 3k+1 split (tile 2k rows 85-127 = f0-42, tile 2k+1
    rows 0-41 = f43-84), block 3k+2 at tile 2k+1 rows 42-126.  Same 11
    tiles (bytes) per 16-block group as the old SLOT=88 layout but only
    5 tile-crossing blocks instead of 10 -> 21 instead of 26 M=64 L1
    pair-streams per group.  Each instr: f = r - shift, one merged M=64
    (outs 0-50 + 13 zero cols), crossing parts accumulate with
    IDENTICAL output APs (mixed-shape accumulation groups are toxic).
    """
    plan = []
    for i in range(BLKS_PER_GRP):
        k, c = i // 3, i % 3
        if c == 0:
            plan.append([(2 * k, 0, True, True)])
        elif c == 1:
            plan.append([(2 * k, 85, True, False),
                         (2 * k + 1, -43, False, True)])
        else:
            plan.append([(2 * k + 1, 42, True, True)])
    return plan


_L1PLAN = _l1_plan()
_L1COLS = 64 * sum(len(v) for v in _L1PLAN)   # wl1 cols (M=64 slots)


def _w1val(W1, f, m):
    j, k = f // 5, f % 5
    j2, o = m // 3, m % 3
    return float(W1[o, k]) if j == j2 and m <= 50 else 0.0


def _host_packs(W1, b1, W2, b2, W3, b3, Wmu, bmu, Wlv, blv):
    # L1 weights: [128, mw] per part-instr, packed along cols
    wl1 = np.zeros((128, _L1COLS), np.float32)
    off = 0
    for i in range(BLKS_PER_GRP):
        for _t, shift, _, _ in _L1PLAN[i]:
            blk = np.zeros((128, 64), np.float32)
            for r in range(128):
                f = r - shift
                if 0 <= f <= 84:
                    for m in range(51):
                        blk[r, m] = _w1val(W1, f, m)
            wl1[:, off:off + 64] = blk
            off += 64

    # L2: [128, 64]: h1 row 64*par+r (r<51) -> col 32*par+mo = W2[mo, r]
    wl2 = np.zeros((128, 64), np.float32)
    for par in range(2):
        for r in range(51):
            for mo in range(32):
                wl2[64 * par + r, 32 * par + mo] = W2[mo, r]

    # L3: [128, 64]: h2 rows 64a+32d+q -> cols 32a+16d+o = W3[o, q]
    wl3 = np.zeros((128, 64), np.float32)
    for a in range(2):
        for d in range(2):
            for q in range(32):
                for o in range(16):
                    wl3[64 * a + 32 * d + q, 32 * a + 16 * d + o] = W3[o, q]

    # heads: [128, 64]: h3 rows 64a+16d+q -> cols 32a+6d+o6 (cols
    # 32a+24..32a+31 zero so the M=64 instr fully covers its rows)
    Wh = np.concatenate([Wmu, Wlv], axis=0)          # [6, 16]
    wlh = np.zeros((128, 64), np.float32)
    for a in range(2):
        for d in range(4):
            for q in range(16):
                for o in range(6):
                    wlh[64 * a + 16 * d + q, 32 * a + 6 * d + o] = Wh[o, q]

    wpack = np.concatenate([wl1, wl2, wl3, wlh], axis=1).astype(BF16_NP)

    # bias pack [128, 4]
    b1v = np.zeros(128, np.float32)
    for half in range(2):
        for r in range(32):
            b1v[64 * half + r] = b1[r % 3]
        for r in range(19):
            b1v[64 * half + 32 + r] = b1[(32 + r) % 3]
    b2v = np.tile(b2, 4).astype(np.float32)
    b3v = np.tile(np.concatenate([b3, b3]), 4)[:128].astype(np.float32)
    bh6 = np.concatenate([bmu, blv])
    bhv = np.zeros(128, np.float32)
    for s in range(4):
        for k in range(24):
            bhv[32 * s + k] = bh6[k % 6]
    bpack = np.stack([b1v, b2v, np.asarray(b3v), bhv],
                     axis=1).astype(np.float32)     # [128, 4]
    return wpack, bpack


_OFF_L2 = _L1COLS
_OFF_L3 = _OFF_L2 + 64
_OFF_H = _OFF_L3 + 64
_WCOLS = _OFF_H + 64

_G_OFF = [g * GCOLS for g in range(15)] + [15 * GCOLS]
_G_COLS = [GCOLS] * 15 + [TCOLS]


def _prep_core(x_flat, c_flat):
    """[n,51]+[n,34] fp32 -> stream layout [128, XCOLS] bf16."""
    n = x_flat.shape[0]
    X = np.zeros((NF, NTOT), np.float32)
    xr = x_flat.reshape(n, 17, 3)
    cr = c_flat.reshape(n, 17, 2)
    for j in range(17):
        X[5 * j:5 * j + 3, :n] = xr[:, j, :].T
        X[5 * j + 3:5 * j + 5, :n] = cr[:, j, :].T
    Xb = X.astype(BF16_NP)
    xs = np.zeros((128, XCOLS), BF16_NP)
    # 3-blocks-per-2-tiles packing: block start row 256*(i//3)+85*(i%3)
    # full groups
    Xf = Xb[:, :15 * 16 * 512].reshape(NF, 15, BLKS_PER_GRP, 512)
    xsf = np.zeros((128, 15, 11, 512), BF16_NP)
    for i in range(BLKS_PER_GRP):
        s = 256 * (i // 3) + 85 * (i % 3)
        for f in range(NF):
            rr = s + f
            xsf[rr % 128, :, rr // 128, :] = Xf[f, :, i, :]
    xs[:, :15 * GCOLS] = xsf.reshape(128, 15 * GCOLS)
    # tail group (8 blocks, 6 tiles)
    Xt = Xb[:, 15 * 16 * 512:].reshape(NF, TAIL_BLKS, 512)
    xst = np.zeros((128, 6, 512), BF16_NP)
    for i in range(TAIL_BLKS):
        s = 256 * (i // 3) + 85 * (i % 3)
        for f in range(NF):
            rr = s + f
            xst[rr % 128, rr // 128, :] = Xt[f, i, :]
    xs[:, 15 * GCOLS:] = xst.reshape(128, TCOLS)
    return np.ascontiguousarray(xs)


def _unpack_out(od):
    """od [128, NOUT*512] bf16 -> (mu, lv) [NOUT*16*512, 3] fp32."""
    arr = np.asarray(od, np.float32).reshape(4, 32, NOUT, 512)[:, :24]
    arr = arr.reshape(4, 4, 6, NOUT, 512)            # s, d, o6, g, c
    arr = np.transpose(arr, (3, 0, 1, 4, 2)).reshape(NOUT * 16 * 512, 6)
    return arr[:, 0:3], arr[:, 3:6]


# --- walrus sync-wait-limit workaround -----------------------------------
_ws_ctr = [0]


def _split_excess_waits(nc, max_waits=1):
    for fn in nc.m.functions:
        for bb in fn.blocks:
            insts = bb.instructions
            i = 0
            while i < len(insts):
                inst = insts[i]
                si = inst.sync_info
                if si is None or si.on_wait is None or \
                        len(si.on_wait) <= max_waits:
                    i += 1
                    continue
                waits = list(si.on_wait)
                keep = waits[-max_waits:]
                excess = waits[:-max_waits]
                new_nops = []
                for w in excess:
                    _ws_ctr[0] += 1
                    nop = mybir.InstNoOp(
                        name=f"I-waitsplit-{_ws_ctr[0]}",
                        sync_info=mybir.SyncInfo(on_wait=[w], on_update=[]),
                        bass_nofuse=True,
                        engine=inst.engine,
                    )
                    new_nops.append(nop)
                inst.sync_info = mybir.SyncInfo(
                    on_wait=keep, on_update=list(si.on_update or []))
                for j, nop in enumerate(new_nops):
                    insts.insert(i + j, nop)
                i += len(new_nops) + 1


def build_kernel(ngrp=NGRP, repeat=1, mode="full"):
    l1_slots = []       # per phase: list of (col_off, tile_idx, st, sp)
    off = 0
    for i in range(BLKS_PER_GRP):
        slots = []
        for t_idx, _shift, st, sp in _L1PLAN[i]:
            slots.append((off, t_idx, st, sp))
            off += 64
        l1_slots.append(slots)

    nc = bass.Bass("TRN2")
    xd = nc.dram_tensor("xs", [128, XCOLS], BF16, kind="ExternalInput")
    wd = nc.dram_tensor("wpack", [128, _WCOLS], BF16, kind="ExternalInput")
    bd = nc.dram_tensor("bpack", [128, 4], F32, kind="ExternalInput")
    od = nc.dram_tensor("out_dev", [128, NOUT * 512], BF16,
                        kind="ExternalOutput")

    with tile.TileContext(nc) as tc:
        with tc.tile_pool(name="const", bufs=1) as constp, \
             tc.tile_pool(name="x", bufs=3 if mode == "pf3" else 2) as xp, \
             tc.tile_pool(name="h1", bufs=8 if mode == "h8" else 6) as h1p, \
             tc.tile_pool(name="h2",
                          bufs=5 if mode == "slack" else 4) as h2p, \
             tc.tile_pool(name="h3",
                          bufs=4 if mode == "slack" else 3) as h3p, \
             tc.tile_pool(name="osb",
                          bufs=3 if mode == "slack" else 2) as osbp, \
             tc.tile_pool(name="ps1", bufs=2, space="PSUM") as ps1p, \
             tc.tile_pool(name="ps2", bufs=2, space="PSUM") as ps2p, \
             tc.tile_pool(name="ps3", bufs=1, space="PSUM") as ps3p, \
             tc.tile_pool(name="ps4", bufs=1, space="PSUM") as ps4p:

            wt = constp.tile([128, _WCOLS], BF16)
            bt = constp.tile([128, 4], F32)
            # ACT ring: don't head-block the first x chunks on SP's FIFO
            nc.scalar.dma_start(out=wt, in_=wd[:, :])
            nc.scalar.dma_start(out=bt, in_=bd[:, :])
            b1v = bt[:, 0:1]
            b2v = bt[:, 1:2]
            b3v = bt[:, 2:3]
            bhv = bt[:, 3:4]

            # cost-weighted DVE/ACT assignment for psum->sbuf passes
            vt = [0.0, 0.0]

            def vpass(pp, h, bias, relu=True, cols=512):
                if mode == "minpass":
                    pp, h = pp[:, 0:32], h[:, 0:32]
                cd = cols / 0.96 + 125.0
                ca = cols / 1.2 + 185.0
                if mode == "dveonly":
                    eng = 0
                elif mode == "actonly":
                    eng = 1
                elif mode == "odve" and not relu:
                    # keep ACT's stream pure-Relu: an Identity pass on
                    # ACT may trigger activation-table set switches
                    eng = 0
                else:
                    eng = 0 if vt[0] + cd <= vt[1] + ca else 1
                if eng == 0:
                    vt[0] += cd
                    nc.vector.tensor_scalar(
                        out=h, in0=pp, scalar1=bias, scalar2=0.0,
                        op0=ALU.add, op1=ALU.max if relu else ALU.bypass)
                else:
                    vt[1] += ca
                    nc.scalar.activation(
                        h, pp, AF.Relu if relu else AF.Identity, bias=bias)

            ps1_t, h1_t, ps2_t, h2_t = {}, {}, {}, {}
            ps3_t, h3_t, ps4_t = {}, {}, {}
            stc = {"L2": 0, "relu2": 0, "L3": 0, "relu3": 0,
                   "heads": 0, "out": 0}

            def relu1(u):                 # u = pair-pair index (4 blocks)
                h1 = h1p.tile([128, 1024], BF16, name="h1t")
                h1_t[u] = h1
                vpass(ps1_t.pop(u), h1, b1v, cols=1024)

            def emit_L2(p):
                assert stc["L2"] == p
                stc["L2"] += 1
                q = p // 2
                if q not in ps2_t:
                    ps2_t[q] = ps2p.tile([128, 512], F32, name="ps2t")
                half = 64 * (p % 2)
                h1w = h1_t[p // 2]
                nc.tensor.matmul(
                    ps2_t[q][half:half + 64, :],
                    wt[:, _OFF_L2:_OFF_L2 + 64],
                    h1w[:, 512 * (p % 2):512 * (p % 2) + 512],
                    start=True, stop=True,
                    tile_position=(0, half))
                if p % 2 == 1 and p // 2 >= 1:
                    h1_t.pop(p // 2 - 1, None)

            def emit_relu2(q):
                assert stc["relu2"] == q
                stc["relu2"] += 1
                h2 = h2p.tile([128, 512], BF16, name="h2t")
                h2_t[q] = h2
                vpass(ps2_t.pop(q), h2, b2v)

            def emit_L3(q):
                assert stc["L3"] == q
                stc["L3"] += 1
                o = q // 2
                if o not in ps3_t:
                    ps3_t[o] = ps3p.tile([128, 512], F32, name="ps3t")
                half = 64 * (q % 2)
                nc.tensor.matmul(
                    ps3_t[o][half:half + 64, :],
                    wt[:, _OFF_L3:_OFF_L3 + 64],
                    h2_t[q], start=True, stop=True,
                    tile_position=(0, half))
                if q >= 1:
                    h2_t.pop(q - 1, None)

            def emit_relu3(o):
                assert stc["relu3"] == o
                stc["relu3"] += 1
                h3 = h3p.tile([128, 512], BF16, name="h3t")
                h3_t[o] = h3
                vpass(ps3_t.pop(o), h3, b3v)

            def emit_heads(o):
                assert stc["heads"] == o
                stc["heads"] += 1
                w = o // 2
                if w not in ps4_t:
                    ps4_t[w] = ps4p.tile([128, 512], F32, name="ps4t")
                half = 64 * (o % 2)
                nc.tensor.matmul(
                    ps4_t[w][half:half + 64, :],
                    wt[:, _OFF_H:_OFF_H + 64],
                    h3_t[o], start=True, stop=True,
                    tile_position=(0, half))
                if o >= 1:
                    h3_t.pop(o - 1, None)

            def emit_out(w):
                assert stc["out"] == w
                stc["out"] += 1
                osb = osbp.tile([128, 512], BF16, name="osbt")
                vpass(ps4_t.pop(w), osb, bhv, relu=False)
                nc.sync.dma_start(
                    out=od[:, 512 * (w % NOUT):512 * (w % NOUT) + 512],
                    in_=osb)

            N = repeat * NBLK
            NG_TOT = repeat * 16          # groups incl. tails
            xt_t = {}

            def fetch_x(gabs):
                g = gabs % 16
                ncols = _G_COLS[g]
                xt_t[gabs] = xp.tile([128, ncols], BF16, name="xt")
                if mode != "nodma":
                    off = _G_OFF[g]
                    # first group: 4 chunks so L1 starts after ~1/4 group
                    nch = 4 if gabs == 0 else 2
                    cw = 512 * ((ncols // 512 + nch - 1) // nch)
                    c0 = 0
                    while c0 < ncols:
                        cw_i = min(cw, ncols - c0)
                        nc.sync.dma_start(
                            out=xt_t[gabs][:, c0:c0 + cw_i],
                            in_=xd[:, off + c0:off + c0 + cw_i])
                        c0 += cw_i
                else:
                    nc.vector.memset(
                        xt_t[gabs][:, 0:2].bitcast(mybir.dt.uint32), 0)

            if mode == "dmaonly":
                for gabs in range(NG_TOT):
                    fetch_x(gabs)
                    g = gabs % 16
                    half = _G_COLS[g] // 2
                    h = h1p.tile([128, 4], BF16, name="h1t")
                    nc.vector.tensor_copy(out=h[:, 0:2],
                                          in_=xt_t[gabs][:, 0:2])
                    nc.vector.tensor_copy(out=h[:, 2:4],
                                          in_=xt_t[gabs][:, half:half + 2])
                    xt_t.pop(gabs - 2, None)
                N = 0

            xt = None
            for t in range(N):
                bb = t % NBLK
                gabs = (t // NBLK) * 16 + bb // BLKS_PER_GRP
                i = bb % BLKS_PER_GRP
                if i == 0:
                    if gabs not in xt_t:
                        fetch_x(gabs)
                    xt_t.pop(gabs - 2, None)
                    xt = xt_t[gabs]
                if i == 2 and gabs + 1 < NG_TOT:
                    fetch_x(gabs + 1)
                if mode == "pf3" and i == 8 and gabs + 2 < NG_TOT \
                        and gabs + 2 not in xt_t:
                    fetch_x(gabs + 2)
                par = t % 2
                u = t // 4
                if t % 4 == 0:
                    ps1_t[u] = ps1p.tile([128, 1024], F32, name="ps1t")
                ps1 = ps1_t[u]
                coff = 512 * ((t // 2) % 2)
                base = 64 * par
                for coloff, t_idx, st, sp in l1_slots[i]:
                    rhs = xt[:, 512 * t_idx:512 * t_idx + 512]
                    nc.tensor.matmul(
                        ps1[base:base + 64, coff:coff + 512],
                        wt[:, coloff:coloff + 64],
                        rhs, start=st, stop=sp,
                        tile_position=(0, base))
                if t % 4 == 3:
                    relu1(u)
                    # lag u-units before L2: PE work between the wide
                    # relu1 pass issue and the L2 that waits on it.
                    # Interleaved A/B on HW: lag2 > lag1 by 2.9us,
                    # lag3 > lag2 by 3.5us, lag4 > lag3 by 3.5us,
                    # lag5 CLIFFS (-23us: h1 ring saturates and pass
                    # engines block on PE buffer-release).
                    lag = {"lag1": 1, "lag2": 2, "lag3": 3,
                           "lag5": 5}.get(mode, 4)
                    # tail taper: drain ~2 units/iter over the last lag
                    # iterations so the serial flush tail shrinks to one
                    # unit (only affects single-shot ramp-down; steady-
                    # state emission is identical)
                    ulast = N // 4 - 1
                    adv_to = u - lag
                    if u > ulast - lag:
                        adv_to = min(u - 1,
                                     u - lag + 2 * (u - (ulast - lag)))
                    while stc["relu2"] <= adv_to:
                        q = stc["relu2"]
                        emit_L2(2 * q)
                        emit_L2(2 * q + 1)
                        emit_relu2(q)
                        if q >= 1:
                            q3 = q - 1
                            emit_L3(q3)
                            if q3 % 2 == 1:
                                o = q3 // 2
                                emit_relu3(o)
                                if o >= 1:
                                    emit_heads(o - 1)
                                    if (o - 1) % 2 == 1:
                                        emit_out((o - 1) // 2)
            # pipeline flush (in dependency order)
            P, Q, O, W = N // 2, N // 4, N // 8, (N + 15) // 16
            while stc["L2"] < P:
                emit_L2(stc["L2"])
            while stc["relu2"] < Q:
                emit_relu2(stc["relu2"])
            while stc["L3"] < Q:
                emit_L3(stc["L3"])
            while stc["relu3"] < O:
                emit_relu3(stc["relu3"])
            while stc["heads"] < O:
                emit_heads(stc["heads"])
            while stc["out"] < W:
                emit_out(stc["out"])

    # walrus hard-caps sync waits at 1 per instruction (max_waits=2
    # fails codegen); the 262 split-NoOps/pass (88 on PE) are forced
    _split_excess_waits(nc)
    return nc


_NC_CACHE = {}


def _get_nc(ngrp=NGRP, repeat=1, mode="full"):
    key = (ngrp, repeat, mode)
    if key not in _NC_CACHE:
        _NC_CACHE[key] = build_kernel(ngrp, repeat, mode)
    return _NC_CACHE[key]


def kernel(x, c, W1, b1, W2, b2, W3, b3, Wmu, bmu, Wlv, blv, _trace=False):
    x = np.asarray(x, np.float32).reshape(B_FULL, 51)
    c = np.asarray(c, np.float32).reshape(B_FULL, 34)
    wpack, bpack = _host_packs(
        np.asarray(W1, np.float32), np.asarray(b1, np.float32),
        np.asarray(W2, np.float32), np.asarray(b2, np.float32),
        np.asarray(W3, np.float32), np.asarray(b3, np.float32),
        np.asarray(Wmu, np.float32), np.asarray(bmu, np.float32),
        np.asarray(Wlv, np.float32), np.asarray(blv, np.float32))

    in_maps = []
    for core in range(N_CORES):
        sl = slice(core * PER_CORE, (core + 1) * PER_CORE)
        in_maps.append({"xs": _prep_core(x[sl], c[sl]),
                        "wpack": wpack, "bpack": bpack})

    nc = _get_nc()
    res = run_bass_kernel_spmd(nc, in_maps, core_ids=list(range(N_CORES)),
                               trace=_trace)
    mus, lvs = [], []
    for i in range(N_CORES):
        mu_i, lv_i = _unpack_out(res.results[i]["out_dev"])
        mus.append(mu_i[:PER_CORE])
        lvs.append(lv_i[:PER_CORE])
    out = (np.concatenate(mus), np.concatenate(lvs))
    if _trace:
        return out, res
    return out
